# revision 13
# baseline (speedup 1.0000x reference)
"""Batched multi-head graph attention (GAT) kernel for 8 Trainium2 NeuronCores.

Math (per batch b, head h):
    hp      = h[b] @ w[h]                          # [N, F]
    t       = tanh(hp)
    s       = t @ a_src[h];  d = t @ a_dst[h]      # [N]
    score   = leaky_relu(s_i + d_j, 0.2)
    e       = where(adj>0, exp(score), 0)
    out     = (e / e.sum(-1, keepdim)) @ hp + bias

On-device identities (row-constant factors cancel in softmax):
    exp(leaky(z)) = max(exp(z), exp(0.2 z))
    D[j,i] = adj[i,j] * max(q_i v_j, v2_j),  q=e^{0.8s}, v=e^d, v2=e^{0.2d}
    max(qv, v2) = v2 + relu(qv - v2)  (the "act" construct: relu on the
    Scalar engine + a v2-scaled [hp|1] matmul against adjT restores the v2
    part exactly, since adjT >= 0)

adj mask trick: adj values are exactly 0.0/1.0 fp32; the high 16 bits read
as fp16 0 / 1.875 -- a constant scale on surviving terms that cancels in
the normalization.  The host sends the high u16 halves of adj^T
(pre-transposed + key-rotated), so the device does plain DMAs.

Sharding: 8 cores = 4 batches x 2 query-row halves; each core handles all 4
heads for its 1024 query rows against all 2048 keys (keys rotated on host so
the core's queries are local rows [0, 1024)).

Engine balance: the N x N elementwise work (score construct + mask) is
spread across DVE (dual-op tensor_scalar / tensor_tensor), Scalar (relu
constructs, with PE fixup matmuls), and GpSimd, with per-(head, block)
assignment knobs.  Setup of pair (2,3) overlaps the main loop of pair (0,1).
"""

import os
from contextlib import ExitStack

import numpy as np

import concourse.bass as bass
import concourse.mybir as mybir
import concourse.tile as tile
from concourse import bacc
from concourse.bass_utils import run_bass_kernel_spmd
from concourse.masks import make_identity

F32 = mybir.dt.float32
F16 = mybir.dt.float16
U16 = mybir.dt.uint16
ALU = mybir.AluOpType
ACTF = mybir.ActivationFunctionType
AX = mybir.AxisListType

B, N, H, F = 4, 2048, 4, 64
NCORES = 8
ROWS = N // 2          # query rows per core
KEYS = N               # keys per core (full)
NEG_SLOPE = 0.2


def default_construct(h, jb):
    """Engine for the score-construct of tile (head h, key-block jb).

    "dve": max(q*v, v2) via one dual-op tensor_scalar on DVE
    "gps": same op on GpSimd
    "act": relu(q*v - v2) on Scalar; needs the v2 fixup matmul
    """
    if h % 2 == 1:
        return "dve" if jb % 4 == 3 else "act"
    else:
        return "dve" if jb % 4 == 1 else "gps"


def default_mask(h, jb):
    """Engine for the mask multiply of tile (h, jb): "dve" or "gps"."""
    return "dve"


def default_spill(i):
    return "act" if i % 2 == 0 else "dve"


def build_program(rows=ROWS, keys=KEYS, heads=H, f=F,
                  construct=default_construct, mask=default_mask,
                  spill=default_spill, sums_f16=True):
    nc = bacc.Bacc("TRN2", target_bir_lowering=False, debug=False)

    kb = keys // 128          # key blocks
    qb = rows // 128          # query blocks
    nhalf = rows // 512       # psum accumulator column halves
    fe = f + 1                # hp with ones column appended

    hb_d = nc.dram_tensor("hb", [keys, f], F32, kind="ExternalInput")
    adjt_d = nc.dram_tensor("adjt", [keys, rows], U16, kind="ExternalInput")
    w_d = nc.dram_tensor("wmat", [heads, f, f], F32, kind="ExternalInput")
    ap_d = nc.dram_tensor("apairt", [heads, 2, f], F32, kind="ExternalInput")
    out_d = nc.dram_tensor("out", [heads, rows, f], F32,
                           kind="ExternalOutput")

    pairs = [tuple(range(p, min(p + 2, heads))) for p in range(0, heads, 2)]
    nq = 512 // 128  # transpose chunks per acc tile

    with tile.TileContext(nc) as tc:
        with (
            tc.tile_pool(name="const", bufs=1) as const,
            tc.tile_pool(name="persist", bufs=1) as persist,
            tc.tile_pool(name="stmp", bufs=2) as stmp,
            tc.tile_pool(name="adjp", bufs=kb) as adjp,
            tc.tile_pool(name="ep", bufs=6) as ep,
            tc.tile_pool(name="dp", bufs=6) as dp,
            tc.tile_pool(name="outp", bufs=4) as outp,
        ):
            # ---- input DMAs first: adjT blocks are the long pole ---------
            adjts = []
            for jb in range(kb):
                adjt = adjp.tile([128, rows], U16, tag="adjt",
                                 name=f"adjt{jb}")
                nc.sync.dma_start(
                    out=adjt, in_=adjt_d.ap()[jb * 128:(jb + 1) * 128, :])
                adjts.append(adjt)

            h32 = persist.tile([128, kb, f], F32, tag="h32")
            nc.sync.dma_start(
                out=h32, in_=hb_d.ap().rearrange("(t p) f -> p t f", p=128))
            w32 = persist.tile([f, heads, f], F32, tag="w32")
            nc.sync.dma_start(out=w32, in_=w_d.ap().rearrange("h f o -> f h o"))
            apr32 = persist.tile([1, heads, 2, f], F32, tag="apr32")
            nc.sync.dma_start(out=apr32, in_=ap_d.ap().unsqueeze(0))

            id16 = const.tile([128, 128], F16, tag="id16")
            make_identity(nc, id16)
            ones16 = const.tile([1, 128], F16, tag="ones16")
            nc.vector.memset(ones16, 1.0)

            h16 = persist.tile([128, kb, f], F16, tag="h16")
            nc.vector.tensor_copy(h16, h32)
            w16 = persist.tile([f, heads, f], F16, tag="w16")
            nc.vector.tensor_copy(w16, w32)
            apr16 = persist.tile([1, heads * 2 * f], F16, tag="apr16")
            nc.vector.tensor_copy(
                apr16, apr32.rearrange("p h t o -> p (h t o)"))

            # a16: [128, H, 2, f] broadcast of a_src/a_dst rows via PE
            a16 = persist.tile([128, heads, 2, f], F16, tag="a16")
            with tc.tile_pool(name="psum_bc0", bufs=1, space="PSUM") as pbc0:
                pa = pbc0.tile([128, heads * 2 * f], F32, tag="pa")
                nc.tensor.matmul(pa, lhsT=ones16, rhs=apr16,
                                 start=True, stop=True)
                nc.scalar.activation(
                    a16, pa.rearrange("p (h t o) -> p h t o", h=heads, t=2),
                    ACTF.Identity)

            # ---- hT (transposed h, fp16) --------------------------------
            hT16 = persist.tile([64, keys], F16, tag="hT16")
            g_ht = min(4, kb)
            with tc.tile_pool(name="psum_ht", bufs=2, space="PSUM") as pht:
                for g in range(kb // g_ht):
                    pt = pht.tile([64, g_ht * 128], F16, tag="pht")
                    for t in range(g_ht):
                        blk = g * g_ht + t
                        nc.tensor.transpose(
                            pt[:, t * 128:(t + 1) * 128],
                            h16[:, blk, :], id16)
                    nc.scalar.activation(
                        hT16[:, g * g_ht * 128:(g + 1) * g_ht * 128], pt,
                        ACTF.Identity)

            # ---- persistent per-head tiles ------------------------------
            hpt = [persist.tile([128, kb, fe], F16, tag=f"hpt{h}",
                                name=f"hpt{h}")
                   for h in range(heads)]
            hpt2 = {}
            qbc = [persist.tile([128, rows], F16, tag=f"qb{h}",
                                name=f"qb{h}")
                   for h in range(heads)]
            vv = [persist.tile([128, kb], F32, tag=f"v{h}", name=f"v{h}")
                  for h in range(heads)]
            vv2 = [persist.tile([128, kb], F32, tag=f"v2{h}", name=f"v2{h}")
                   for h in range(heads)]
            nvv2 = {}
            for h in range(heads):
                if any(construct(h, jb) == "act" for jb in range(kb)):
                    hpt2[h] = persist.tile([128, kb, fe], F16,
                                           tag=f"hpt2{h}", name=f"hpt2{h}")
                    nvv2[h] = persist.tile([128, kb], F32, tag=f"nv2{h}",
                                           name=f"nv2{h}")

            acc_sb = persist.tile([fe, heads * nhalf, 512], F16, tag="acc_sb")

            def setup_head(h, S):
                """Emit per-head setup. S: dict of psum pools."""
                php, pqb = S["php"], S["pqb"]
                # hp matmuls in 2 groups of 8 blocks
                tanh_h = stmp.tile([128, kb, f], F16, tag="tanh",
                                   name=f"tanh{h}")
                g_hp = min(8, kb)
                for k in range(kb // g_hp):
                    pp = php.tile([128, g_hp * f], F32, tag="php")
                    for t in range(g_hp):
                        blk = k * g_hp + t
                        nc.tensor.matmul(
                            pp[:, t * f:(t + 1) * f],
                            lhsT=hT16[:, blk * 128:(blk + 1) * 128],
                            rhs=w16[:, h, :], start=True, stop=True)
                    nc.scalar.activation(
                        hpt[h][:, k * g_hp:(k + 1) * g_hp, 0:f],
                        pp.rearrange("p (t o) -> p t o", o=f),
                        ACTF.Identity)
                    nc.scalar.activation(
                        tanh_h[:, k * g_hp:(k + 1) * g_hp, :],
                        pp.rearrange("p (t o) -> p t o", o=f),
                        ACTF.Tanh)
                nc.vector.memset(hpt[h][:, :, f:fe], 1.0)

                # s (first qb blocks) and d (all blocks) via mul + reduce
                prod = stmp.tile([128, kb, 2, f], F16, tag="prod",
                                 name=f"prod{h}")
                nc.vector.tensor_tensor(
                    out=prod[:, 0:qb, 0:1, :],
                    in0=tanh_h[:, 0:qb, :].unsqueeze(2),
                    in1=a16[:, h, 0:1, :].unsqueeze(1).broadcast_to(
                        [128, qb, 1, f]),
                    op=ALU.mult)
                nc.vector.tensor_tensor(
                    out=prod[:, :, 1:2, :],
                    in0=tanh_h.unsqueeze(2),
                    in1=a16[:, h, 1:2, :].unsqueeze(1).broadcast_to(
                        [128, kb, 1, f]),
                    op=ALU.mult)
                if sums_f16:
                    sums = stmp.tile([128, kb, 2], F16, tag="sums",
                                     name=f"sums{h}")
                    with nc.allow_low_precision(reason="64-elt dot in fp16"):
                        nc.vector.reduce_sum(sums[:, 0:qb, 0:1],
                                             prod[:, 0:qb, 0:1, :], axis=AX.X)
                        nc.vector.reduce_sum(sums[:, :, 1:2],
                                             prod[:, :, 1:2, :], axis=AX.X)
                else:
                    sums = stmp.tile([128, kb, 2], F32, tag="sums",
                                     name=f"sums{h}")
                    nc.vector.reduce_sum(sums[:, 0:qb, 0:1],
                                         prod[:, 0:qb, 0:1, :], axis=AX.X)
                    nc.vector.reduce_sum(sums[:, :, 1:2],
                                         prod[:, :, 1:2, :], axis=AX.X)

                nc.scalar.activation(vv[h], sums[:, :, 1], ACTF.Exp)
                nc.scalar.activation(vv2[h], sums[:, :, 1], ACTF.Exp,
                                     scale=NEG_SLOPE)
                if h in nvv2:
                    nc.vector.tensor_scalar_mul(nvv2[h], vv2[h], -1.0)
                    nc.vector.tensor_tensor(
                        out=hpt2[h], in0=hpt[h],
                        in1=vv2[h].unsqueeze(2).broadcast_to([128, kb, fe]),
                        op=ALU.mult)

                # q = exp(0.8 s): exp in block layout, transpose to a row,
                # then broadcast down the partitions with a ones matmul
                qsm = stmp.tile([128, qb], F16, tag="qsm", name=f"qsm{h}")
                nc.scalar.activation(qsm, sums[:, 0:qb, 0], ACTF.Exp,
                                     scale=1.0 - NEG_SLOPE)
                ptq = S["ptq"].tile([1, rows], F16, tag="ptq")
                for t in range(qb):
                    nc.tensor.transpose(
                        ptq[:, t * 128:(t + 1) * 128],
                        qsm[:, t:t + 1], id16)
                qrow = stmp.tile([1, rows], F16, tag="qrow", name=f"qrow{h}")
                nc.vector.tensor_copy(qrow, ptq)
                for gq in range(rows // 512):
                    sl = slice(gq * 512, (gq + 1) * 512)
                    pq = pqb.tile([128, 512], F32, tag="pqb")
                    nc.tensor.matmul(pq, lhsT=ones16, rhs=qrow[:, sl],
                                     start=True, stop=True)
                    if h % 2 == 0:
                        nc.scalar.activation(qbc[h][:, sl], pq, ACTF.Identity)
                    else:
                        nc.vector.tensor_copy(qbc[h][:, sl], pq)

            def pair_loop(pair, accs):
                for jb in range(kb):
                    adj16 = adjts[jb].bitcast(F16)
                    for h in pair:
                        c = construct(h, jb)
                        v_s = vv[h][:, jb:jb + 1]
                        v2_s = vv2[h][:, jb:jb + 1]
                        ea = ep.tile([128, rows], F16, tag="ea",
                                     name=f"ea{h}_{jb}")
                        if c == "act":
                            nc.scalar.activation(
                                ea, qbc[h], ACTF.Relu,
                                bias=nvv2[h][:, jb:jb + 1], scale=v_s)
                        elif c == "gps":
                            nc.gpsimd.tensor_scalar(
                                out=ea, in0=qbc[h],
                                scalar1=v_s, scalar2=v2_s,
                                op0=ALU.mult, op1=ALU.max)
                        else:
                            nc.vector.tensor_scalar(
                                out=ea, in0=qbc[h],
                                scalar1=v_s, scalar2=v2_s,
                                op0=ALU.mult, op1=ALU.max)
                        da = dp.tile([128, rows], F16, tag="da",
                                     name=f"da{h}_{jb}")
                        m_eng = (nc.gpsimd if mask(h, jb) == "gps"
                                 else nc.vector)
                        m_eng.tensor_tensor(out=da, in0=ea, in1=adj16,
                                            op=ALU.mult)
                        is_act = c == "act"
                        for half in range(nhalf):
                            sl = slice(half * 512, (half + 1) * 512)
                            nc.tensor.matmul(
                                accs[h % 2 * nhalf + half],
                                lhsT=hpt[h][:, jb, :],
                                rhs=da[:, sl],
                                start=(jb == 0),
                                stop=(jb == kb - 1 and not is_act))
                            if is_act:
                                nc.tensor.matmul(
                                    accs[h % 2 * nhalf + half],
                                    lhsT=hpt2[h][:, jb, :],
                                    rhs=adj16[:, sl],
                                    start=False, stop=(jb == kb - 1))

            def pair_spill(pair, accs):
                for h in pair:
                    for half in range(nhalf):
                        i = h * nhalf + half
                        a = accs[h % 2 * nhalf + half]
                        if spill(i) == "act":
                            nc.scalar.activation(
                                acc_sb[:, i, :], a, ACTF.Identity)
                        else:
                            nc.vector.tensor_copy(acc_sb[:, i, :], a)

            def pair_norm(pair, ptf):
                for h in pair:
                    for half in range(nhalf):
                        i = h * nhalf + half
                        fpad = fe + 3  # pad so fp16 q-stride stays 4B-aligned
                        pt = ptf.tile([128, nq, fpad], F16, tag="ptf")
                        for q in range(nq):
                            nc.tensor.transpose(
                                pt[:, q, 0:fe],
                                acc_sb[:, i, q * 128:(q + 1) * 128],
                                id16[0:fe, 0:fe])
                        rcol = outp.tile([128, nq], F16, tag="rcol")
                        with nc.allow_low_precision(reason="fp16 1/den"):
                            nc.vector.reciprocal(rcol, pt[:, :, f])
                        osb = outp.tile([128, nq, f], F32, tag="osb")
                        nc.vector.tensor_tensor(
                            out=osb, in0=pt[:, :, 0:f],
                            in1=rcol.unsqueeze(2).broadcast_to([128, nq, f]),
                            op=ALU.mult)
                        nc.sync.dma_start(
                            out=out_d.ap()[
                                h, half * 512:(half + 1) * 512, :]
                            .rearrange("(q p) f -> p q f", p=128),
                            in_=osb)

            # ---- pipelined schedule -------------------------------------
            with ExitStack() as sched:
                s0 = ExitStack()
                S0 = {
                    "php": s0.enter_context(
                        tc.tile_pool(name="php0", bufs=1, space="PSUM")),
                    "pqb": s0.enter_context(
                        tc.tile_pool(name="pqb0", bufs=2, space="PSUM")),
                    "ptq": s0.enter_context(
                        tc.tile_pool(name="ptq0", bufs=1, space="PSUM")),
                }
                for h in pairs[0]:
                    setup_head(h, S0)
                s0.close()

                a0 = ExitStack()
                accp0 = a0.enter_context(
                    tc.tile_pool(name="accp0", bufs=1, space="PSUM"))
                accs0 = [accp0.tile([fe, 512], F32, tag=f"acc0_{i}",
                                    name=f"acc0_{i}")
                         for i in range(2 * nhalf)]

                # pair1 setup (overlaps pair0 loop; emitted before the loop)
                s1 = ExitStack()
                S1 = {
                    "php": s1.enter_context(
                        tc.tile_pool(name="php1", bufs=1, space="PSUM")),
                    "pqb": s1.enter_context(
                        tc.tile_pool(name="pqb1", bufs=2, space="PSUM")),
                    "ptq": s1.enter_context(
                        tc.tile_pool(name="ptq1", bufs=1, space="PSUM")),
                }
                for h in pairs[1]:
                    setup_head(h, S1)

                pair_loop(pairs[0], accs0)
                pair_spill(pairs[0], accs0)
                s1.close()
                a0.close()

                a1 = ExitStack()
                accp1 = a1.enter_context(
                    tc.tile_pool(name="accp1", bufs=1, space="PSUM"))
                accs1 = [accp1.tile([fe, 512], F32, tag=f"acc1_{i}",
                                    name=f"acc1_{i}")
                         for i in range(2 * nhalf)]

                ptf0 = ExitStack()
                pt0 = ptf0.enter_context(
                    tc.tile_pool(name="ptf0", bufs=2, space="PSUM"))
                pair_norm(pairs[0], pt0)

                pair_loop(pairs[1], accs1)
                pair_spill(pairs[1], accs1)
                ptf0.close()

                ptf1 = ExitStack()
                pt1 = ptf1.enter_context(
                    tc.tile_pool(name="ptf1", bufs=2, space="PSUM"))
                pair_norm(pairs[1], pt1)
                ptf1.close()
                a1.close()
    nc.compile()
    return nc


_PROGRAM_CACHE = {}


def _get_program():
    key = "full"
    if key not in _PROGRAM_CACHE:
        _PROGRAM_CACHE[key] = build_program()
    return _PROGRAM_CACHE[key]


def make_in_maps(h, adj, w, a_src, a_dst):
    """Shard + marshal the full inputs into 8 per-core input maps."""
    h = np.ascontiguousarray(np.asarray(h, dtype=np.float32))
    adj = np.asarray(adj, dtype=np.float32)
    w = np.ascontiguousarray(np.asarray(w, dtype=np.float32))
    apairt = np.ascontiguousarray(
        np.concatenate([np.asarray(a_src)[:, None, :, 0],
                        np.asarray(a_dst)[:, None, :, 0]],
                       axis=1).astype(np.float32))  # [H, 2, F]
    in_maps = []
    for c in range(NCORES):
        b, r0 = c // 2, (c % 2) * ROWS
        hb = np.concatenate([h[b, r0:], h[b, :r0]], axis=0)  # rotate keys
        adjT = adj[b].T  # [keys, rows-full]
        adjt_rot = np.concatenate(
            [adjT[r0:, r0:r0 + ROWS], adjT[:r0, r0:r0 + ROWS]], axis=0)
        adjt_u16 = np.ascontiguousarray(
            np.ascontiguousarray(adjt_rot).view(np.uint16)
            .reshape(KEYS, ROWS, 2)[:, :, 1])
        in_maps.append({
            "hb": np.ascontiguousarray(hb),
            "adjt": adjt_u16,
            "wmat": w,
            "apairt": apairt,
        })
    return in_maps


def assemble_output(results, bias):
    """Gather per-core [H, ROWS, F] results into [B, H, N, F]."""
    out = np.empty((B, H, N, F), dtype=np.float32)
    for c in range(NCORES):
        b, r0 = c // 2, (c % 2) * ROWS
        out[b, :, r0:r0 + ROWS, :] = results[c]["out"]
    if bias is not None:
        out = out + np.asarray(bias, dtype=np.float32)[None, None, None, :]
    return out


def run(h, adj, w, a_src, a_dst, bias, trace=False, trace_kwargs=None):
    nc = _get_program()
    in_maps = make_in_maps(h, adj, w, a_src, a_dst)
    res = run_bass_kernel_spmd(nc, in_maps, core_ids=list(range(NCORES)),
                               trace=trace, **(trace_kwargs or {}))
    return assemble_output(res.results, bias), res


def kernel(h, adj, w, a_src, a_dst, bias):
    out, _ = run(h, adj, w, a_src, a_dst, bias,
                 trace=bool(int(os.environ.get("GAT_TRACE", "0"))))
    return out


# revision 15
# speedup vs baseline: 3.0604x; 3.0604x over previous
"""Batched multi-head graph attention (GAT) kernel for 8 Trainium2 NeuronCores.

Math (per batch b, head h):
    hp      = h[b] @ w[h]                          # [N, F]
    t       = tanh(hp)
    s       = t @ a_src[h];  d = t @ a_dst[h]      # [N]
    score   = leaky_relu(s_i + d_j, 0.2)
    e       = where(adj>0, exp(score), 0)
    out     = (e / e.sum(-1, keepdim)) @ hp + bias

On-device identities (row-constant factors cancel in softmax):
    exp(leaky(z)) = max(exp(z), exp(0.2 z))
    D[j,i] = adj[i,j] * max(q_i v_j, v2_j),  q=e^{0.8s}, v=e^d, v2=e^{0.2d}
    max(qv, v2) = v2 + relu(qv - v2)  (the "act" construct: relu on the
    Scalar engine + a v2-scaled [hp|1] matmul against adjT restores the v2
    part exactly, since adjT >= 0)

adj mask trick: adj values are exactly 0.0/1.0 fp32; the high 16 bits read
as fp16 0 / 1.875 -- a constant scale on surviving terms that cancels in
the normalization.  The host sends the high u16 halves of adj^T
(pre-transposed + key-rotated), so the device does plain DMAs.

Sharding: 8 cores = 4 batches x 2 query-row halves; each core handles all 4
heads for its 1024 query rows against all 2048 keys (keys rotated on host so
the core's queries are local rows [0, 1024)).

Engine balance: the N x N elementwise work (score construct + mask) is
spread across DVE (dual-op tensor_scalar / tensor_tensor), Scalar (relu
constructs, with PE fixup matmuls), and GpSimd, with per-(head, block)
assignment knobs.  Setup of pair (2,3) overlaps the main loop of pair (0,1).
"""

import os
from contextlib import ExitStack

import numpy as np

import concourse.bass as bass
import concourse.mybir as mybir
import concourse.tile as tile
from concourse import bacc
from concourse.bass_utils import run_bass_kernel_spmd
from concourse.masks import make_identity

F32 = mybir.dt.float32
F16 = mybir.dt.float16
U16 = mybir.dt.uint16
ALU = mybir.AluOpType
ACTF = mybir.ActivationFunctionType
AX = mybir.AxisListType

B, N, H, F = 4, 2048, 4, 64
NCORES = 8
ROWS = N // 2          # query rows per core
KEYS = N               # keys per core (full)
NEG_SLOPE = 0.2


def default_construct(h, jb):
    """Engine for the score-construct of tile (head h, key-block jb).

    "dve": max(q*v, v2) via one dual-op tensor_scalar on DVE
    "gps": same op on GpSimd
    "act": relu(q*v - v2) on Scalar; needs the v2 fixup matmul
    """
    return "act" if h % 2 == 1 else "dve"


def default_mask(h, jb):
    """Engine for the mask multiply of tile (h, jb): "dve" or "gps"."""
    return "gps" if (h == 0 and jb % 4 == 2) else "dve"


def default_spill(i):
    return "act" if i % 2 == 0 else "dve"


def build_program(rows=ROWS, keys=KEYS, heads=H, f=F,
                  construct=default_construct, mask=default_mask,
                  spill=default_spill, sums_f16=True):
    nc = bacc.Bacc("TRN2", target_bir_lowering=False, debug=False)

    kb = keys // 128          # key blocks
    qb = rows // 128          # query blocks
    nhalf = rows // 512       # psum accumulator column halves
    fe = f + 1                # hp with ones column appended

    hb_d = nc.dram_tensor("hb", [keys, f], F32, kind="ExternalInput")
    adjt_d = nc.dram_tensor("adjt", [keys, rows], U16, kind="ExternalInput")
    w_d = nc.dram_tensor("wmat", [heads, f, f], F32, kind="ExternalInput")
    ap_d = nc.dram_tensor("apairt", [heads, 2, f], F32, kind="ExternalInput")
    out_d = nc.dram_tensor("out", [heads, rows, f], F32,
                           kind="ExternalOutput")

    pairs = [tuple(range(p, min(p + 2, heads))) for p in range(0, heads, 2)]
    nq = 512 // 128  # transpose chunks per acc tile

    with tile.TileContext(nc) as tc:
        with (
            tc.tile_pool(name="const", bufs=1) as const,
            tc.tile_pool(name="persist", bufs=1) as persist,
            tc.tile_pool(name="stmp", bufs=2) as stmp,
            tc.tile_pool(name="adjp", bufs=kb) as adjp,
            tc.tile_pool(name="ep", bufs=6) as ep,
            tc.tile_pool(name="dp", bufs=6) as dp,
            tc.tile_pool(name="outp", bufs=4) as outp,
        ):
            # ---- input DMAs first: adjT blocks are the long pole ---------
            adjts = []
            for jb in range(kb):
                adjt = adjp.tile([128, rows], U16, tag="adjt",
                                 name=f"adjt{jb}")
                nc.sync.dma_start(
                    out=adjt, in_=adjt_d.ap()[jb * 128:(jb + 1) * 128, :])
                adjts.append(adjt)

            h32 = persist.tile([128, kb, f], F32, tag="h32")
            nc.sync.dma_start(
                out=h32, in_=hb_d.ap().rearrange("(t p) f -> p t f", p=128))
            w32 = persist.tile([f, heads, f], F32, tag="w32")
            nc.sync.dma_start(out=w32, in_=w_d.ap().rearrange("h f o -> f h o"))
            apr32 = persist.tile([1, heads, 2, f], F32, tag="apr32")
            nc.sync.dma_start(out=apr32, in_=ap_d.ap().unsqueeze(0))

            id16 = const.tile([128, 128], F16, tag="id16")
            make_identity(nc, id16)
            ones16 = const.tile([1, 128], F16, tag="ones16")
            nc.vector.memset(ones16, 1.0)

            h16 = persist.tile([128, kb, f], F16, tag="h16")
            nc.vector.tensor_copy(h16, h32)
            w16 = persist.tile([f, heads, f], F16, tag="w16")
            nc.vector.tensor_copy(w16, w32)
            apr16 = persist.tile([1, heads * 2 * f], F16, tag="apr16")
            nc.vector.tensor_copy(
                apr16, apr32.rearrange("p h t o -> p (h t o)"))

            # a16: [128, H, 2, f] broadcast of a_src/a_dst rows via PE
            a16 = persist.tile([128, heads, 2, f], F16, tag="a16")
            with tc.tile_pool(name="psum_bc0", bufs=1, space="PSUM") as pbc0:
                pa = pbc0.tile([128, heads * 2 * f], F32, tag="pa")
                nc.tensor.matmul(pa, lhsT=ones16, rhs=apr16,
                                 start=True, stop=True)
                nc.scalar.activation(
                    a16, pa.rearrange("p (h t o) -> p h t o", h=heads, t=2),
                    ACTF.Identity)

            # ---- hT (transposed h, fp16) --------------------------------
            hT16 = persist.tile([64, keys], F16, tag="hT16")
            g_ht = min(4, kb)
            with tc.tile_pool(name="psum_ht", bufs=2, space="PSUM") as pht:
                for g in range(kb // g_ht):
                    pt = pht.tile([64, g_ht * 128], F16, tag="pht")
                    for t in range(g_ht):
                        blk = g * g_ht + t
                        nc.tensor.transpose(
                            pt[:, t * 128:(t + 1) * 128],
                            h16[:, blk, :], id16)
                    nc.scalar.activation(
                        hT16[:, g * g_ht * 128:(g + 1) * g_ht * 128], pt,
                        ACTF.Identity)

            # ---- persistent per-head tiles ------------------------------
            hpt = [persist.tile([128, kb, fe], F16, tag=f"hpt{h}",
                                name=f"hpt{h}")
                   for h in range(heads)]
            hpt2 = {}
            qbc = [persist.tile([128, rows], F16, tag=f"qb{h}",
                                name=f"qb{h}")
                   for h in range(heads)]
            vv = [persist.tile([128, kb], F32, tag=f"v{h}", name=f"v{h}")
                  for h in range(heads)]
            vv2 = [persist.tile([128, kb], F32, tag=f"v2{h}", name=f"v2{h}")
                   for h in range(heads)]
            nvv2 = {}
            for h in range(heads):
                if any(construct(h, jb) == "act" for jb in range(kb)):
                    hpt2[h] = persist.tile([128, kb, fe], F16,
                                           tag=f"hpt2{h}", name=f"hpt2{h}")
                    nvv2[h] = persist.tile([128, kb], F32, tag=f"nv2{h}",
                                           name=f"nv2{h}")

            acc_sb = persist.tile([fe, heads * nhalf, 512], F16, tag="acc_sb")

            def setup_head(h, S):
                """Emit per-head setup. S: dict of psum pools."""
                php, pqb = S["php"], S["pqb"]
                # hp matmuls in 2 groups of 8 blocks
                tanh_h = stmp.tile([128, kb, f], F16, tag="tanh",
                                   name=f"tanh{h}")
                g_hp = min(8, kb)
                for k in range(kb // g_hp):
                    pp = php.tile([128, g_hp * f], F32, tag="php")
                    for t in range(g_hp):
                        blk = k * g_hp + t
                        nc.tensor.matmul(
                            pp[:, t * f:(t + 1) * f],
                            lhsT=hT16[:, blk * 128:(blk + 1) * 128],
                            rhs=w16[:, h, :], start=True, stop=True)
                    nc.scalar.activation(
                        hpt[h][:, k * g_hp:(k + 1) * g_hp, 0:f],
                        pp.rearrange("p (t o) -> p t o", o=f),
                        ACTF.Identity)
                    nc.scalar.activation(
                        tanh_h[:, k * g_hp:(k + 1) * g_hp, :],
                        pp.rearrange("p (t o) -> p t o", o=f),
                        ACTF.Tanh)
                nc.vector.memset(hpt[h][:, :, f:fe], 1.0)

                # s (first qb blocks) and d (all blocks) via mul + reduce
                prod = stmp.tile([128, kb, 2, f], F16, tag="prod",
                                 name=f"prod{h}")
                nc.vector.tensor_tensor(
                    out=prod[:, 0:qb, 0:1, :],
                    in0=tanh_h[:, 0:qb, :].unsqueeze(2),
                    in1=a16[:, h, 0:1, :].unsqueeze(1).broadcast_to(
                        [128, qb, 1, f]),
                    op=ALU.mult)
                nc.vector.tensor_tensor(
                    out=prod[:, :, 1:2, :],
                    in0=tanh_h.unsqueeze(2),
                    in1=a16[:, h, 1:2, :].unsqueeze(1).broadcast_to(
                        [128, kb, 1, f]),
                    op=ALU.mult)
                if sums_f16:
                    sums = stmp.tile([128, kb, 2], F16, tag="sums",
                                     name=f"sums{h}")
                    with nc.allow_low_precision(reason="64-elt dot in fp16"):
                        nc.vector.reduce_sum(sums[:, 0:qb, 0:1],
                                             prod[:, 0:qb, 0:1, :], axis=AX.X)
                        nc.vector.reduce_sum(sums[:, :, 1:2],
                                             prod[:, :, 1:2, :], axis=AX.X)
                else:
                    sums = stmp.tile([128, kb, 2], F32, tag="sums",
                                     name=f"sums{h}")
                    nc.vector.reduce_sum(sums[:, 0:qb, 0:1],
                                         prod[:, 0:qb, 0:1, :], axis=AX.X)
                    nc.vector.reduce_sum(sums[:, :, 1:2],
                                         prod[:, :, 1:2, :], axis=AX.X)

                nc.scalar.activation(vv[h], sums[:, :, 1], ACTF.Exp)
                nc.scalar.activation(vv2[h], sums[:, :, 1], ACTF.Exp,
                                     scale=NEG_SLOPE)
                if h in nvv2:
                    nc.vector.tensor_scalar_mul(nvv2[h], vv2[h], -1.0)
                    nc.vector.tensor_tensor(
                        out=hpt2[h], in0=hpt[h],
                        in1=vv2[h].unsqueeze(2).broadcast_to([128, kb, fe]),
                        op=ALU.mult)

                # q = exp(0.8 s): exp in block layout, transpose to a row,
                # then broadcast down the partitions with a ones matmul
                qsm = stmp.tile([128, qb], F16, tag="qsm", name=f"qsm{h}")
                nc.scalar.activation(qsm, sums[:, 0:qb, 0], ACTF.Exp,
                                     scale=1.0 - NEG_SLOPE)
                ptq = S["ptq"].tile([1, rows], F16, tag="ptq")
                for t in range(qb):
                    nc.tensor.transpose(
                        ptq[:, t * 128:(t + 1) * 128],
                        qsm[:, t:t + 1], id16)
                qrow = stmp.tile([1, rows], F16, tag="qrow", name=f"qrow{h}")
                nc.vector.tensor_copy(qrow, ptq)
                for gq in range(rows // 512):
                    sl = slice(gq * 512, (gq + 1) * 512)
                    pq = pqb.tile([128, 512], F32, tag="pqb")
                    nc.tensor.matmul(pq, lhsT=ones16, rhs=qrow[:, sl],
                                     start=True, stop=True)
                    if h % 2 == 0:
                        nc.scalar.activation(qbc[h][:, sl], pq, ACTF.Identity)
                    else:
                        nc.vector.tensor_copy(qbc[h][:, sl], pq)

            def pair_loop(pair, accs):
                np_ = len(pair)
                for jb in range(kb):
                    adj16 = adjts[jb].bitcast(F16)
                    ea = ep.tile([128, np_, rows], F16, tag="ea",
                                 name=f"ea{pair[0]}_{jb}")
                    da = dp.tile([128, np_, rows], F16, tag="da",
                                 name=f"da{pair[0]}_{jb}")
                    for k, h in enumerate(pair):
                        c = construct(h, jb)
                        v_s = vv[h][:, jb:jb + 1]
                        v2_s = vv2[h][:, jb:jb + 1]
                        if c == "act":
                            nc.scalar.activation(
                                ea[:, k, :], qbc[h], ACTF.Relu,
                                bias=nvv2[h][:, jb:jb + 1], scale=v_s)
                        else:
                            nc.vector.tensor_scalar(
                                out=ea[:, k, :], in0=qbc[h],
                                scalar1=v_s, scalar2=v2_s,
                                op0=ALU.mult, op1=ALU.max)
                    m0, m1 = mask(pair[0], jb), mask(pair[1], jb)
                    if m0 == m1 == "dve":
                        nc.vector.tensor_tensor(
                            out=da, in0=ea,
                            in1=adj16.unsqueeze(1).broadcast_to(
                                [128, np_, rows]),
                            op=ALU.mult)
                    else:
                        for k, h in enumerate(pair):
                            m_eng = (nc.gpsimd if mask(h, jb) == "gps"
                                     else nc.vector)
                            m_eng.tensor_tensor(
                                out=da[:, k, :], in0=ea[:, k, :],
                                in1=adj16, op=ALU.mult)
                    for k, h in enumerate(pair):
                        is_act = construct(h, jb) == "act"
                        for half in range(nhalf):
                            sl = slice(half * 512, (half + 1) * 512)
                            nc.tensor.matmul(
                                accs[h % 2 * nhalf + half],
                                lhsT=hpt[h][:, jb, :],
                                rhs=da[:, k, sl],
                                start=(jb == 0),
                                stop=(jb == kb - 1 and not is_act))
                            if is_act:
                                nc.tensor.matmul(
                                    accs[h % 2 * nhalf + half],
                                    lhsT=hpt2[h][:, jb, :],
                                    rhs=adj16[:, sl],
                                    start=False, stop=(jb == kb - 1))

            def pair_spill(pair, accs):
                for h in pair:
                    for half in range(nhalf):
                        i = h * nhalf + half
                        a = accs[h % 2 * nhalf + half]
                        if spill(i) == "act":
                            nc.scalar.activation(
                                acc_sb[:, i, :], a, ACTF.Identity)
                        else:
                            nc.vector.tensor_copy(acc_sb[:, i, :], a)

            def pair_norm(pair, ptf):
                for h in pair:
                    for half in range(nhalf):
                        i = h * nhalf + half
                        fpad = fe + 3  # pad so fp16 q-stride stays 4B-aligned
                        pt = ptf.tile([128, nq, fpad], F16, tag="ptf")
                        for q in range(nq):
                            nc.tensor.transpose(
                                pt[:, q, 0:fe],
                                acc_sb[:, i, q * 128:(q + 1) * 128],
                                id16[0:fe, 0:fe])
                        rcol = outp.tile([128, nq], F16, tag="rcol")
                        with nc.allow_low_precision(reason="fp16 1/den"):
                            nc.vector.reciprocal(rcol, pt[:, :, f])
                        osb = outp.tile([128, nq, f], F32, tag="osb")
                        nc.vector.tensor_tensor(
                            out=osb, in0=pt[:, :, 0:f],
                            in1=rcol.unsqueeze(2).broadcast_to([128, nq, f]),
                            op=ALU.mult)
                        nc.sync.dma_start(
                            out=out_d.ap()[
                                h, half * 512:(half + 1) * 512, :]
                            .rearrange("(q p) f -> p q f", p=128),
                            in_=osb)

            # ---- pipelined schedule -------------------------------------
            with ExitStack() as sched:
                s0 = ExitStack()
                S0 = {
                    "php": s0.enter_context(
                        tc.tile_pool(name="php0", bufs=1, space="PSUM")),
                    "pqb": s0.enter_context(
                        tc.tile_pool(name="pqb0", bufs=2, space="PSUM")),
                    "ptq": s0.enter_context(
                        tc.tile_pool(name="ptq0", bufs=1, space="PSUM")),
                }
                for h in pairs[0]:
                    setup_head(h, S0)
                s0.close()

                a0 = ExitStack()
                accp0 = a0.enter_context(
                    tc.tile_pool(name="accp0", bufs=1, space="PSUM"))
                accs0 = [accp0.tile([fe, 512], F32, tag=f"acc0_{i}",
                                    name=f"acc0_{i}")
                         for i in range(2 * nhalf)]

                # pair1 setup (overlaps pair0 loop; emitted before the loop)
                s1 = ExitStack()
                S1 = {
                    "php": s1.enter_context(
                        tc.tile_pool(name="php1", bufs=1, space="PSUM")),
                    "pqb": s1.enter_context(
                        tc.tile_pool(name="pqb1", bufs=2, space="PSUM")),
                    "ptq": s1.enter_context(
                        tc.tile_pool(name="ptq1", bufs=1, space="PSUM")),
                }
                for h in pairs[1]:
                    setup_head(h, S1)

                pair_loop(pairs[0], accs0)
                pair_spill(pairs[0], accs0)
                s1.close()
                a0.close()

                a1 = ExitStack()
                accp1 = a1.enter_context(
                    tc.tile_pool(name="accp1", bufs=1, space="PSUM"))
                accs1 = [accp1.tile([fe, 512], F32, tag=f"acc1_{i}",
                                    name=f"acc1_{i}")
                         for i in range(2 * nhalf)]

                ptf0 = ExitStack()
                pt0 = ptf0.enter_context(
                    tc.tile_pool(name="ptf0", bufs=2, space="PSUM"))
                pair_norm(pairs[0], pt0)

                pair_loop(pairs[1], accs1)
                pair_spill(pairs[1], accs1)
                ptf0.close()

                ptf1 = ExitStack()
                pt1 = ptf1.enter_context(
                    tc.tile_pool(name="ptf1", bufs=2, space="PSUM"))
                pair_norm(pairs[1], pt1)
                ptf1.close()
                a1.close()
    nc.compile()
    return nc


_PROGRAM_CACHE = {}


def _get_program():
    key = "full"
    if key not in _PROGRAM_CACHE:
        _PROGRAM_CACHE[key] = build_program()
    return _PROGRAM_CACHE[key]


def make_in_maps(h, adj, w, a_src, a_dst):
    """Shard + marshal the full inputs into 8 per-core input maps."""
    h = np.ascontiguousarray(np.asarray(h, dtype=np.float32))
    adj = np.asarray(adj, dtype=np.float32)
    w = np.ascontiguousarray(np.asarray(w, dtype=np.float32))
    apairt = np.ascontiguousarray(
        np.concatenate([np.asarray(a_src)[:, None, :, 0],
                        np.asarray(a_dst)[:, None, :, 0]],
                       axis=1).astype(np.float32))  # [H, 2, F]
    in_maps = []
    for c in range(NCORES):
        b, r0 = c // 2, (c % 2) * ROWS
        hb = np.concatenate([h[b, r0:], h[b, :r0]], axis=0)  # rotate keys
        adjT = adj[b].T  # [keys, rows-full]
        adjt_rot = np.concatenate(
            [adjT[r0:, r0:r0 + ROWS], adjT[:r0, r0:r0 + ROWS]], axis=0)
        adjt_u16 = np.ascontiguousarray(
            np.ascontiguousarray(adjt_rot).view(np.uint16)
            .reshape(KEYS, ROWS, 2)[:, :, 1])
        in_maps.append({
            "hb": np.ascontiguousarray(hb),
            "adjt": adjt_u16,
            "wmat": w,
            "apairt": apairt,
        })
    return in_maps


def assemble_output(results, bias):
    """Gather per-core [H, ROWS, F] results into [B, H, N, F]."""
    out = np.empty((B, H, N, F), dtype=np.float32)
    for c in range(NCORES):
        b, r0 = c // 2, (c % 2) * ROWS
        out[b, :, r0:r0 + ROWS, :] = results[c]["out"]
    if bias is not None:
        out = out + np.asarray(bias, dtype=np.float32)[None, None, None, :]
    return out


def run(h, adj, w, a_src, a_dst, bias, trace=False, trace_kwargs=None):
    nc = _get_program()
    in_maps = make_in_maps(h, adj, w, a_src, a_dst)
    res = run_bass_kernel_spmd(nc, in_maps, core_ids=list(range(NCORES)),
                               trace=trace, **(trace_kwargs or {}))
    return assemble_output(res.results, bias), res


def kernel(h, adj, w, a_src, a_dst, bias):
    out, _ = run(h, adj, w, a_src, a_dst, bias,
                 trace=bool(int(os.environ.get("GAT_TRACE", "0"))))
    return out


# revision 17
# speedup vs baseline: 4.1093x; 1.3427x over previous
"""Batched multi-head graph attention (GAT) kernel for 8 Trainium2 NeuronCores.

Math (per batch b, head h):
    hp      = h[b] @ w[h]                          # [N, F]
    t       = tanh(hp)
    s       = t @ a_src[h];  d = t @ a_dst[h]      # [N]
    score   = leaky_relu(s_i + d_j, 0.2)
    e       = where(adj>0, exp(score), 0)
    out     = (e / e.sum(-1, keepdim)) @ hp + bias

On-device identities (row-constant factors cancel in softmax):
    exp(leaky(z)) = max(exp(z), exp(0.2 z))
    D[j,i] = adj[i,j] * max(q_i v_j, v2_j),  q=e^{0.8s}, v=e^d, v2=e^{0.2d}
    max(qv, v2) = v2 + relu(qv - v2)  (the "act" construct: relu on the
    Scalar engine + a v2-scaled [hp|1] matmul against adjT restores the v2
    part exactly, since adjT >= 0)

adj mask trick: adj values are exactly 0.0/1.0 fp32; the high 16 bits read
as fp16 0 / 1.875 -- a constant scale on surviving terms that cancels in
the normalization.  The host sends the high u16 halves of adj^T
(pre-transposed + key-rotated), so the device does plain DMAs.

Sharding: 8 cores = 4 batches x 2 query-row halves; each core handles all 4
heads for its 1024 query rows against all 2048 keys (keys rotated on host so
the core's queries are local rows [0, 1024)).

Engine balance: the N x N elementwise work (score construct + mask) is
spread across DVE (dual-op tensor_scalar / tensor_tensor), Scalar (relu
constructs, with PE fixup matmuls), and GpSimd, with per-(head, block)
assignment knobs.  Setup of pair (2,3) overlaps the main loop of pair (0,1).
"""

import os
from contextlib import ExitStack

import numpy as np

import concourse.bass as bass
import concourse.mybir as mybir
import concourse.tile as tile
from concourse import bacc
from concourse.bass_utils import run_bass_kernel_spmd
from concourse.masks import make_identity

F32 = mybir.dt.float32
F16 = mybir.dt.float16
U16 = mybir.dt.uint16
ALU = mybir.AluOpType
ACTF = mybir.ActivationFunctionType
AX = mybir.AxisListType

B, N, H, F = 4, 2048, 4, 64
NCORES = 8
ROWS = N // 2          # query rows per core
KEYS = N               # keys per core (full)
NEG_SLOPE = 0.2


def default_construct(h, jb):
    """Engine for the score-construct of tile (head h, key-block jb).

    "dve": max(q*v, v2) via one dual-op tensor_scalar on DVE
    "gps": same op on GpSimd
    "act": relu(q*v - v2) on Scalar; needs the v2 fixup matmul
    """
    return "act" if h % 2 == 1 else "dve"


def default_mask(h, jb):
    """Engine for the mask multiply of tile (h, jb): "dve" or "gps"."""
    return "dve"


def default_spill(i):
    return "act" if i % 2 == 0 else "dve"


def build_program(rows=ROWS, keys=KEYS, heads=H, f=F,
                  construct=default_construct, mask=default_mask,
                  spill=default_spill, sums_f16=True):
    nc = bacc.Bacc("TRN2", target_bir_lowering=False, debug=False)

    kb = keys // 128          # key blocks
    qb = rows // 128          # query blocks
    nhalf = rows // 512       # psum accumulator column halves
    fe = f + 1                # hp with ones column appended

    hb_d = nc.dram_tensor("hb", [keys, f], F32, kind="ExternalInput")
    adjt_d = nc.dram_tensor("adjt", [keys, rows], U16, kind="ExternalInput")
    w_d = nc.dram_tensor("wmat", [heads, f, f], F32, kind="ExternalInput")
    ap_d = nc.dram_tensor("apairt", [heads, 2, f], F32, kind="ExternalInput")
    nhalf_ = rows // 512
    out_d = nc.dram_tensor("out", [heads, nhalf_, 512 // 128, 128, f], F32,
                           kind="ExternalOutput")

    pairs = [tuple(range(p, min(p + 2, heads))) for p in range(0, heads, 2)]
    nq = 512 // 128  # transpose chunks per acc tile

    with tile.TileContext(nc) as tc:
        with (
            tc.tile_pool(name="const", bufs=1) as const,
            tc.tile_pool(name="persist", bufs=1) as persist,
            tc.tile_pool(name="stmp", bufs=2) as stmp,
            tc.tile_pool(name="adjp", bufs=4) as adjp,
            tc.tile_pool(name="ep", bufs=4) as ep,
            tc.tile_pool(name="dp", bufs=4) as dp,
            tc.tile_pool(name="outp", bufs=4) as outp,
        ):
            # ---- input DMAs: small setup tensors first, then the mask ----
            h32 = persist.tile([128, kb, f], F32, tag="h32")
            nc.sync.dma_start(
                out=h32, in_=hb_d.ap().rearrange("(t p) f -> p t f", p=128))
            w32 = persist.tile([f, heads, f], F32, tag="w32")
            nc.sync.dma_start(out=w32, in_=w_d.ap().rearrange("h f o -> f h o"))
            apr32 = persist.tile([1, heads, 2, f], F32, tag="apr32")
            nc.sync.dma_start(out=apr32, in_=ap_d.ap().unsqueeze(0))

            # adjT in two triggers (first blocks land early for the loop)
            adjtiles = []
            g_adj = 4
            for g0 in range(0, kb, g_adj):
                g1 = min(g0 + g_adj, kb)
                at = adjp.tile([128, g1 - g0, rows], U16, tag="adjt",
                               name=f"adjt_g{g0}")
                nc.sync.dma_start(
                    out=at,
                    in_=adjt_d.ap()[g0 * 128:g1 * 128, :]
                    .rearrange("(t p) n -> p t n", p=128))
                adjtiles.append((g0, at))
            adjts = []
            for jb in range(kb):
                g0, at = adjtiles[jb // g_adj]
                adjts.append(at[:, jb - g0, :])

            id16 = const.tile([128, 128], F16, tag="id16")
            make_identity(nc, id16)
            ones16 = const.tile([1, 128], F16, tag="ones16")
            nc.vector.memset(ones16, 1.0)

            h16 = persist.tile([128, kb, f], F16, tag="h16")
            nc.vector.tensor_copy(h16, h32)
            w16 = persist.tile([f, heads, f], F16, tag="w16")
            nc.vector.tensor_copy(w16, w32)
            apr16 = persist.tile([1, heads * 2 * f], F16, tag="apr16")
            nc.vector.tensor_copy(
                apr16, apr32.rearrange("p h t o -> p (h t o)"))

            # a16: [128, H, 2, f] broadcast of a_src/a_dst rows via PE
            a16 = persist.tile([128, heads, 2, f], F16, tag="a16")
            with tc.tile_pool(name="psum_bc0", bufs=1, space="PSUM") as pbc0:
                pa = pbc0.tile([128, heads * 2 * f], F32, tag="pa")
                nc.tensor.matmul(pa, lhsT=ones16, rhs=apr16,
                                 start=True, stop=True)
                nc.scalar.activation(
                    a16, pa.rearrange("p (h t o) -> p h t o", h=heads, t=2),
                    ACTF.Identity)

            # ---- hT (transposed h, fp16) --------------------------------
            hT16 = persist.tile([64, keys], F16, tag="hT16")
            g_ht = min(4, kb)
            with tc.tile_pool(name="psum_ht", bufs=2, space="PSUM") as pht:
                for g in range(kb // g_ht):
                    pt = pht.tile([64, g_ht * 128], F16, tag="pht")
                    for t in range(g_ht):
                        blk = g * g_ht + t
                        nc.tensor.transpose(
                            pt[:, t * 128:(t + 1) * 128],
                            h16[:, blk, :], id16)
                    nc.scalar.activation(
                        hT16[:, g * g_ht * 128:(g + 1) * g_ht * 128], pt,
                        ACTF.Identity)

            # ---- persistent per-head tiles ------------------------------
            hpt = [persist.tile([128, kb, fe], F16, tag=f"hpt{h}",
                                name=f"hpt{h}")
                   for h in range(heads)]
            hpt2 = {}
            qbc = [persist.tile([128, rows], F16, tag=f"qb{h}",
                                name=f"qb{h}")
                   for h in range(heads)]
            vv = [persist.tile([128, kb], F32, tag=f"v{h}", name=f"v{h}")
                  for h in range(heads)]
            vv2 = [persist.tile([128, kb], F32, tag=f"v2{h}", name=f"v2{h}")
                   for h in range(heads)]
            nvv2 = {}
            for h in range(heads):
                if any(construct(h, jb) == "act" for jb in range(kb)):
                    hpt2[h] = persist.tile([128, kb, fe], F16,
                                           tag=f"hpt2{h}", name=f"hpt2{h}")
                    nvv2[h] = persist.tile([128, kb], F32, tag=f"nv2{h}",
                                           name=f"nv2{h}")

            acc_sb = persist.tile([fe, heads * nhalf, 512], F16, tag="acc_sb")

            def setup_head(h, S):
                """Emit per-head setup. S: dict of psum pools."""
                php, pqb = S["php"], S["pqb"]
                # hp matmuls in 2 groups of 8 blocks
                tanh_h = stmp.tile([128, kb, f], F16, tag="tanh",
                                   name=f"tanh{h}")
                g_hp = min(8, kb)
                for k in range(kb // g_hp):
                    pp = php.tile([128, g_hp * f], F32, tag="php")
                    for t in range(g_hp):
                        blk = k * g_hp + t
                        nc.tensor.matmul(
                            pp[:, t * f:(t + 1) * f],
                            lhsT=hT16[:, blk * 128:(blk + 1) * 128],
                            rhs=w16[:, h, :], start=True, stop=True)
                    nc.scalar.activation(
                        hpt[h][:, k * g_hp:(k + 1) * g_hp, 0:f],
                        pp.rearrange("p (t o) -> p t o", o=f),
                        ACTF.Identity)
                    nc.scalar.activation(
                        tanh_h[:, k * g_hp:(k + 1) * g_hp, :],
                        pp.rearrange("p (t o) -> p t o", o=f),
                        ACTF.Tanh)
                nc.vector.memset(hpt[h][:, :, f:fe], 1.0)

                # s (first qb blocks) and d (all blocks) via mul + reduce
                prod = stmp.tile([128, kb, 2, f], F16, tag="prod",
                                 name=f"prod{h}")
                nc.vector.tensor_tensor(
                    out=prod[:, 0:qb, 0:1, :],
                    in0=tanh_h[:, 0:qb, :].unsqueeze(2),
                    in1=a16[:, h, 0:1, :].unsqueeze(1).broadcast_to(
                        [128, qb, 1, f]),
                    op=ALU.mult)
                nc.vector.tensor_tensor(
                    out=prod[:, :, 1:2, :],
                    in0=tanh_h.unsqueeze(2),
                    in1=a16[:, h, 1:2, :].unsqueeze(1).broadcast_to(
                        [128, kb, 1, f]),
                    op=ALU.mult)
                if sums_f16:
                    sums = stmp.tile([128, kb, 2], F16, tag="sums",
                                     name=f"sums{h}")
                    with nc.allow_low_precision(reason="64-elt dot in fp16"):
                        nc.vector.reduce_sum(sums[:, 0:qb, 0:1],
                                             prod[:, 0:qb, 0:1, :], axis=AX.X)
                        nc.vector.reduce_sum(sums[:, :, 1:2],
                                             prod[:, :, 1:2, :], axis=AX.X)
                else:
                    sums = stmp.tile([128, kb, 2], F32, tag="sums",
                                     name=f"sums{h}")
                    nc.vector.reduce_sum(sums[:, 0:qb, 0:1],
                                         prod[:, 0:qb, 0:1, :], axis=AX.X)
                    nc.vector.reduce_sum(sums[:, :, 1:2],
                                         prod[:, :, 1:2, :], axis=AX.X)

                nc.scalar.activation(vv[h], sums[:, :, 1], ACTF.Exp)
                nc.scalar.activation(vv2[h], sums[:, :, 1], ACTF.Exp,
                                     scale=NEG_SLOPE)
                if h in nvv2:
                    nc.vector.tensor_scalar_mul(nvv2[h], vv2[h], -1.0)
                    nc.vector.tensor_tensor(
                        out=hpt2[h], in0=hpt[h],
                        in1=vv2[h].unsqueeze(2).broadcast_to([128, kb, fe]),
                        op=ALU.mult)

                # q = exp(0.8 s): exp in block layout, transpose to a row,
                # then broadcast down the partitions with a ones matmul
                qsm = stmp.tile([128, qb], F16, tag="qsm", name=f"qsm{h}")
                nc.scalar.activation(qsm, sums[:, 0:qb, 0], ACTF.Exp,
                                     scale=1.0 - NEG_SLOPE)
                ptq = S["ptq"].tile([1, rows], F16, tag="ptq")
                for t in range(qb):
                    nc.tensor.transpose(
                        ptq[:, t * 128:(t + 1) * 128],
                        qsm[:, t:t + 1], id16)
                qrow = stmp.tile([1, rows], F16, tag="qrow", name=f"qrow{h}")
                nc.vector.tensor_copy(qrow, ptq)
                for gq in range(rows // 512):
                    sl = slice(gq * 512, (gq + 1) * 512)
                    pq = pqb.tile([128, 512], F32, tag="pqb")
                    nc.tensor.matmul(pq, lhsT=ones16, rhs=qrow[:, sl],
                                     start=True, stop=True)
                    if h % 2 == 0:
                        nc.scalar.activation(qbc[h][:, sl], pq, ACTF.Identity)
                    else:
                        nc.vector.tensor_copy(qbc[h][:, sl], pq)

            def pair_loop(pair, accs):
                np_ = len(pair)
                for jb in range(kb):
                    adj16 = adjts[jb].bitcast(F16)
                    ea = ep.tile([128, np_, rows], F16, tag="ea",
                                 name=f"ea{pair[0]}_{jb}")
                    da = dp.tile([128, np_, rows], F16, tag="da",
                                 name=f"da{pair[0]}_{jb}")
                    for k, h in enumerate(pair):
                        c = construct(h, jb)
                        v_s = vv[h][:, jb:jb + 1]
                        v2_s = vv2[h][:, jb:jb + 1]
                        if c == "act":
                            nc.scalar.activation(
                                ea[:, k, :], qbc[h], ACTF.Relu,
                                bias=nvv2[h][:, jb:jb + 1], scale=v_s)
                        else:
                            nc.vector.tensor_scalar(
                                out=ea[:, k, :], in0=qbc[h],
                                scalar1=v_s, scalar2=v2_s,
                                op0=ALU.mult, op1=ALU.max)
                    m0, m1 = mask(pair[0], jb), mask(pair[1], jb)
                    if m0 == m1 == "dve":
                        nc.vector.tensor_tensor(
                            out=da, in0=ea,
                            in1=adj16.unsqueeze(1).broadcast_to(
                                [128, np_, rows]),
                            op=ALU.mult)
                    else:
                        for k, h in enumerate(pair):
                            m_eng = (nc.gpsimd if mask(h, jb) == "gps"
                                     else nc.vector)
                            m_eng.tensor_tensor(
                                out=da[:, k, :], in0=ea[:, k, :],
                                in1=adj16, op=ALU.mult)
                    for k, h in enumerate(pair):
                        is_act = construct(h, jb) == "act"
                        for half in range(nhalf):
                            sl = slice(half * 512, (half + 1) * 512)
                            nc.tensor.matmul(
                                accs[h % 2 * nhalf + half],
                                lhsT=hpt[h][:, jb, :],
                                rhs=da[:, k, sl],
                                start=(jb == 0),
                                stop=(jb == kb - 1 and not is_act))
                            if is_act:
                                nc.tensor.matmul(
                                    accs[h % 2 * nhalf + half],
                                    lhsT=hpt2[h][:, jb, :],
                                    rhs=adj16[:, sl],
                                    start=False, stop=(jb == kb - 1))

            def pair_spill(pair, accs):
                for h in pair:
                    for half in range(nhalf):
                        i = h * nhalf + half
                        a = accs[h % 2 * nhalf + half]
                        if spill(i) == "act":
                            nc.scalar.activation(
                                acc_sb[:, i, :], a, ACTF.Identity)
                        else:
                            nc.vector.tensor_copy(acc_sb[:, i, :], a)

            def pair_norm(pair, ptf):
                for h in pair:
                    for half in range(nhalf):
                        i = h * nhalf + half
                        fpad = fe + 3  # pad so fp16 q-stride stays 4B-aligned
                        pt = ptf.tile([128, nq, fpad], F16, tag="ptf")
                        for q in range(nq):
                            nc.tensor.transpose(
                                pt[:, q, 0:fe],
                                acc_sb[:, i, q * 128:(q + 1) * 128],
                                id16[0:fe, 0:fe])
                        rcol = outp.tile([128, nq], F16, tag="rcol")
                        with nc.allow_low_precision(reason="fp16 1/den"):
                            nc.vector.reciprocal(rcol, pt[:, :, f])
                        osb = outp.tile([128, nq, f], F32, tag="osb")
                        nc.vector.tensor_tensor(
                            out=osb, in0=pt[:, :, 0:f],
                            in1=rcol.unsqueeze(2).broadcast_to([128, nq, f]),
                            op=ALU.mult)
                        nc.sync.dma_start(
                            out=out_d.ap()[h, half]
                            .rearrange("q p f -> p q f"),
                            in_=osb)

            # ---- pipelined schedule -------------------------------------
            with ExitStack() as sched:
                s0 = ExitStack()
                S0 = {
                    "php": s0.enter_context(
                        tc.tile_pool(name="php0", bufs=1, space="PSUM")),
                    "pqb": s0.enter_context(
                        tc.tile_pool(name="pqb0", bufs=2, space="PSUM")),
                    "ptq": s0.enter_context(
                        tc.tile_pool(name="ptq0", bufs=1, space="PSUM")),
                }
                for h in pairs[0]:
                    setup_head(h, S0)
                s0.close()

                a0 = ExitStack()
                accp0 = a0.enter_context(
                    tc.tile_pool(name="accp0", bufs=1, space="PSUM"))
                accs0 = [accp0.tile([fe, 512], F32, tag=f"acc0_{i}",
                                    name=f"acc0_{i}")
                         for i in range(2 * nhalf)]

                # pair1 setup (overlaps pair0 loop; emitted before the loop)
                s1 = ExitStack()
                S1 = {
                    "php": s1.enter_context(
                        tc.tile_pool(name="php1", bufs=1, space="PSUM")),
                    "pqb": s1.enter_context(
                        tc.tile_pool(name="pqb1", bufs=2, space="PSUM")),
                    "ptq": s1.enter_context(
                        tc.tile_pool(name="ptq1", bufs=1, space="PSUM")),
                }
                for h in pairs[1]:
                    setup_head(h, S1)

                pair_loop(pairs[0], accs0)
                pair_spill(pairs[0], accs0)
                s1.close()
                a0.close()

                a1 = ExitStack()
                accp1 = a1.enter_context(
                    tc.tile_pool(name="accp1", bufs=1, space="PSUM"))
                accs1 = [accp1.tile([fe, 512], F32, tag=f"acc1_{i}",
                                    name=f"acc1_{i}")
                         for i in range(2 * nhalf)]

                ptf0 = ExitStack()
                pt0 = ptf0.enter_context(
                    tc.tile_pool(name="ptf0", bufs=2, space="PSUM"))
                pair_norm(pairs[0], pt0)

                pair_loop(pairs[1], accs1)
                pair_spill(pairs[1], accs1)
                ptf0.close()

                ptf1 = ExitStack()
                pt1 = ptf1.enter_context(
                    tc.tile_pool(name="ptf1", bufs=2, space="PSUM"))
                pair_norm(pairs[1], pt1)
                ptf1.close()
                a1.close()
    nc.compile()
    return nc


_PROGRAM_CACHE = {}


def _get_program():
    key = "full"
    if key not in _PROGRAM_CACHE:
        _PROGRAM_CACHE[key] = build_program()
    return _PROGRAM_CACHE[key]


def make_in_maps(h, adj, w, a_src, a_dst):
    """Shard + marshal the full inputs into 8 per-core input maps."""
    h = np.ascontiguousarray(np.asarray(h, dtype=np.float32))
    adj = np.asarray(adj, dtype=np.float32)
    w = np.ascontiguousarray(np.asarray(w, dtype=np.float32))
    apairt = np.ascontiguousarray(
        np.concatenate([np.asarray(a_src)[:, None, :, 0],
                        np.asarray(a_dst)[:, None, :, 0]],
                       axis=1).astype(np.float32))  # [H, 2, F]
    in_maps = []
    for c in range(NCORES):
        b, r0 = c // 2, (c % 2) * ROWS
        hb = np.concatenate([h[b, r0:], h[b, :r0]], axis=0)  # rotate keys
        adjT = adj[b].T  # [keys, rows-full]
        adjt_rot = np.concatenate(
            [adjT[r0:, r0:r0 + ROWS], adjT[:r0, r0:r0 + ROWS]], axis=0)
        adjt_u16 = np.ascontiguousarray(
            np.ascontiguousarray(adjt_rot).view(np.uint16)
            .reshape(KEYS, ROWS, 2)[:, :, 1])
        in_maps.append({
            "hb": np.ascontiguousarray(hb),
            "adjt": adjt_u16,
            "wmat": w,
            "apairt": apairt,
        })
    return in_maps


def assemble_output(results, bias):
    """Gather per-core [H, ROWS, F] results into [B, H, N, F]."""
    out = np.empty((B, H, N, F), dtype=np.float32)
    for c in range(NCORES):
        b, r0 = c // 2, (c % 2) * ROWS
        out[b, :, r0:r0 + ROWS, :] = results[c]["out"].reshape(H, ROWS, F)
    if bias is not None:
        out = out + np.asarray(bias, dtype=np.float32)[None, None, None, :]
    return out


def run(h, adj, w, a_src, a_dst, bias, trace=False, trace_kwargs=None):
    nc = _get_program()
    in_maps = make_in_maps(h, adj, w, a_src, a_dst)
    res = run_bass_kernel_spmd(nc, in_maps, core_ids=list(range(NCORES)),
                               trace=trace, **(trace_kwargs or {}))
    return assemble_output(res.results, bias), res


def kernel(h, adj, w, a_src, a_dst, bias):
    out, _ = run(h, adj, w, a_src, a_dst, bias,
                 trace=bool(int(os.environ.get("GAT_TRACE", "0"))))
    return out
